# revision 2
# baseline (speedup 1.0000x reference)
"""Trainium2 Bass kernel for nn_AttentionModule (dense transformer block).

Computation (per batch element b):
    q = X @ Wq.T ; k = K @ Wk.T ; v = X @ Wv.T        (X=query_input, K=key_input)
    a = softmax((k @ q.T) / sqrt(D), axis=-1)          -> (NK, NQ)
    out = a @ v + K                                    -> (NK, D)

Sharding: data-parallel over batch, one batch element per NeuronCore (B == 8).

q and k never appear individually -- only the Gram product k @ q.T does.
Folding the two projection weights into G = Wq.T @ Wk (host-side weight
pre-pack) turns the score matrix into S.T = X @ (G @ K.T), which removes the
entire q projection from the device:  kg.T = G @ K.T costs the same as the
old k projection, and the score matmul is unchanged with X itself as the
stationary operand.  Device work drops from 15.0 to 12.9 GMAC per core.

Layout strategy: matmul contractions run on the partition axis, so the host
pre-transposes X, K and the weights to feature-major layouts (and rounds them
to bf16 -- partial sums stay fp32 in PSUM, and the residual add of key_input
is done in fp32).  The kernel computes kgT (spilled to DRAM) and v (spilled),
scores in [n_q, n_k] layout with X-blocks stationary, exp on the scalar
engine, the softmax denominator with a ones-vector matmul, and the context
matmul consumes exp(S)T directly as the stationary operand.  The
normalization is folded into the output pass as a fused per-partition
multiply-add on the vector engine.
"""

import numpy as np
import ml_dtypes

import concourse.tile as tile
from concourse import bacc, mybir
from concourse.bass_utils import run_bass_kernel_spmd
from concourse.masks import make_identity

B, NQ, NK, D = 8, 2048, 2048, 1024
P = 128
EB = D // P          # 8 feature blocks
NB = NQ // P         # 16 query-row blocks
MC = 512             # scores chunk width (n_k columns per chunk)
NMC = NK // MC       # 4 chunks
SCALE = 1.0 / float(np.sqrt(np.float32(D)))

F32 = mybir.dt.float32
BF16 = mybir.dt.bfloat16

_CACHE = {}


def _build():
    nc = bacc.Bacc("TRN2", target_bir_lowering=False, debug=False, num_devices=B)

    xT = nc.dram_tensor("xT", [D, NQ], BF16, kind="ExternalInput").ap()
    ktT = nc.dram_tensor("ktT", [D, NK], BF16, kind="ExternalInput").ap()
    knat = nc.dram_tensor("knat", [NK, D], F32, kind="ExternalInput").ap()
    gT = nc.dram_tensor("gT", [D, D], BF16, kind="ExternalInput").ap()
    wvT = nc.dram_tensor("wvT", [D, D], BF16, kind="ExternalInput").ap()
    out = nc.dram_tensor("out", [NK, D], F32, kind="ExternalOutput").ap()

    with tile.TileContext(nc) as tc:
        with (
            tc.tile_pool(name="const", bufs=1) as constp,
            tc.tile_pool(name="xin", bufs=EB) as xinp,
            tc.tile_pool(name="dram", bufs=1, space="DRAM") as dramp,
            tc.tile_pool(name="psum", bufs=1, space="PSUM") as psp,
            tc.tile_pool(name="stage", bufs=12) as stagep,
            tc.tile_pool(name="ktc", bufs=18) as ktcp,
        ):
            ident = constp.tile([1, 1], F32, tag="ident", name="ident")
            make_identity(nc, ident)
            ones = constp.tile([P, 1], BF16, tag="ones", name="ones")
            nc.vector.memset(ones, 1.0)

            kg_sp = dramp.tile([D, NK], BF16, tag="kg_sp", name="kg_sp")
            v_sp = dramp.tile([NQ, D], BF16, tag="v_sp", name="v_sp")

            # X.T feature blocks stay SBUF-resident: moving operand for the
            # v projection in phase 1, stationary operand for scores in
            # phase 2.
            x_in = [xinp.tile([P, NQ], BF16, tag="xin", name="xin")
                    for _ in range(EB)]

            # ---------------- phase 1: projections ----------------
            with (
                tc.tile_pool(name="bigin", bufs=16) as bigp,
                tc.tile_pool(name="wpool", bufs=16) as wp,
            ):
                # -- kgT[e, m] = sum_d gT[d, e] * ktT[d, m]  (spilled to DRAM)
                # gT/ktT are loaded in column halves so the first matmul
                # group only waits on half the bytes (shorter pipeline fill).
                # Loads are emitted in first-consumed order: gT first half,
                # then ktT quarters in consumption order, gT second half last.
                g_h = [[None] * 2 for _ in range(EB)]
                kt_q = [[None] * 4 for _ in range(EB)]
                for db in range(EB):
                    t = wp.tile([P, D // 2], BF16, tag="wh", name="wh", bufs=16)
                    nc.sync.dma_start(
                        out=t, in_=gT[db * P:(db + 1) * P, 0:512]
                    )
                    g_h[db][0] = t
                for q in range(4):
                    for db in range(EB):
                        t = bigp.tile([P, NK // 4], BF16, tag="kth", name="kth", bufs=32)
                        nc.sync.dma_start(
                            out=t,
                            in_=ktT[db * P:(db + 1) * P, q * 512:(q + 1) * 512],
                        )
                        kt_q[db][q] = t
                for db in range(EB):
                    t = wp.tile([P, D // 2], BF16, tag="wh", name="wh", bufs=16)
                    nc.sync.dma_start(
                        out=t, in_=gT[db * P:(db + 1) * P, 512:1024]
                    )
                    g_h[db][1] = t
                gi = 0
                for h2 in range(2):
                    for mc4 in range(NK // 512):
                        for eb in range(h2 * 4, h2 * 4 + 4):
                            tg = "mm" if gi % 2 == 0 else "st"
                            gi += 1
                            ps = psp.tile([P, 512], F32, tag=tg, name="mm",
                                          bufs=3 if tg == "mm" else 4)
                            for db in range(EB):
                                nc.tensor.matmul(
                                    ps,
                                    g_h[db][h2][:, (eb % 4) * P:(eb % 4 + 1) * P],
                                    kt_q[db][mc4],
                                    start=(db == 0),
                                    stop=(db == EB - 1),
                                )
                            st = stagep.tile([P, 512], BF16, tag="stage", name="stage")
                            nc.vector.tensor_copy(st, ps)
                            nc.scalar.dma_start(
                                out=kg_sp[eb * P:(eb + 1) * P, mc4 * 512:(mc4 + 1) * 512],
                                in_=st,
                            )

                # prefetch chunk-0 score operands while the v phase runs
                ktc0 = []
                for eb in range(EB):
                    t = ktcp.tile([P, MC], BF16, tag="ktc", name="ktc")
                    nc.sync.dma_start(out=t, in_=kg_sp[eb * P:(eb + 1) * P, 0:MC])
                    ktc0.append(t)

                # -- v[n, dv] = sum_d xT[d, n] * wvT[d, dv]  (spilled to DRAM)
                for db in range(EB):
                    nc.sync.dma_start(out=x_in[db], in_=xT[db * P:(db + 1) * P, :])
                wv = []
                for db in range(EB):
                    t = wp.tile([P, D], BF16, tag="w", name="w", bufs=16)
                    nc.sync.dma_start(out=t, in_=wvT[db * P:(db + 1) * P, :])
                    wv.append(t)
                for nb in range(NB):
                    for dc in range(D // 512):
                        tg = "mm" if (nb * 2 + dc) % 2 == 0 else "st"
                        ps = psp.tile([P, 512], F32, tag=tg, name="mm",
                                      bufs=3 if tg == "mm" else 4)
                        for db in range(EB):
                            nc.tensor.matmul(
                                ps,
                                x_in[db][:, nb * P:(nb + 1) * P],
                                wv[db][:, dc * 512:(dc + 1) * 512],
                                start=(db == 0),
                                stop=(db == EB - 1),
                            )
                        st = stagep.tile([P, 512], BF16, tag="stage", name="stage")
                        nc.vector.tensor_copy(st, ps)
                        nc.scalar.dma_start(
                            out=v_sp[nb * P:(nb + 1) * P, dc * 512:(dc + 1) * 512],
                            in_=st,
                        )

            # ---------------- phase 2: attention ----------------
            with (
                tc.tile_pool(name="expst", bufs=18) as expp,
                tc.tile_pool(name="vst", bufs=20) as vstp,
                tc.tile_pool(name="knp", bufs=6) as knp,
                tc.tile_pool(name="outp", bufs=6) as outp,
                tc.tile_pool(name="small", bufs=4) as smallp,
            ):
                for mc in range(NMC):
                    m0 = mc * MC
                    if mc == 0:
                        ktc = ktc0
                    else:
                        ktc = []
                        for eb in range(EB):
                            t = ktcp.tile([P, MC], BF16, tag="ktc", name="ktc")
                            nc.sync.dma_start(
                                out=t, in_=kg_sp[eb * P:(eb + 1) * P, m0:m0 + MC]
                            )
                            ktc.append(t)

                    # scores + exp + column-sum accumulation
                    expst = []
                    cs_ps = psp.tile([1, MC], F32, tag="csrp", name="cs", bufs=1)
                    for nb in range(NB):
                        st_ps = psp.tile([P, MC], F32, tag="st", name="st", bufs=4)
                        for eb in range(EB):
                            nc.tensor.matmul(
                                st_ps,
                                x_in[eb][:, nb * P:(nb + 1) * P],
                                ktc[eb],
                                start=(eb == 0),
                                stop=(eb == EB - 1),
                            )
                        et = expp.tile([P, MC], BF16, tag="expst", name="expst")
                        nc.scalar.activation(
                            out=et, in_=st_ps,
                            func=mybir.ActivationFunctionType.Exp, scale=SCALE,
                        )
                        expst.append(et)
                        # the column-sum matmul for block j is emitted two
                        # groups late so the exp -> cs semaphore never gates PE
                        if nb >= 2:
                            j = nb - 2
                            nc.tensor.matmul(
                                cs_ps, ones, expst[j],
                                start=(j == 0), stop=False,
                            )

                    for j in (NB - 2, NB - 1):
                        nc.tensor.matmul(
                            cs_ps, ones, expst[j],
                            start=False, stop=(j == NB - 1),
                        )
                    recip_row = smallp.tile([1, MC], F32, tag="rrow", name="rrow")
                    nc.vector.reciprocal(recip_row, cs_ps)
                    rp_ps = psp.tile([P, MC // P], F32, tag="csrp", name="rp", bufs=1)
                    for j in range(MC // P):
                        nc.tensor.transpose(
                            rp_ps[:, j:j + 1],
                            recip_row[:, j * P:(j + 1) * P],
                            ident,
                        )
                    recip_pp = smallp.tile([P, MC // P], F32, tag="rpp", name="rpp")
                    nc.vector.tensor_copy(recip_pp, rp_ps)

                    # context: C[m, dv] = sum_n expst[n, m] * v[n, dv]
                    vts = []
                    for nb in range(NB):
                        vt = vstp.tile([P, D], BF16, tag="vst", name="vst")
                        nc.sync.dma_start(
                            out=vt, in_=v_sp[nb * P:(nb + 1) * P, :],
                        )
                        vts.append(vt)
                    for msb in range(MC // P):
                        r0 = m0 + msb * P
                        kn = knp.tile([P, D], F32, tag="knat", name="knat")
                        nc.sync.dma_start(out=kn, in_=knat[r0:r0 + P, :])
                        ot = outp.tile([P, D], F32, tag="ostage", name="ostage")
                        for dc in range(D // 512):
                            c_ps = psp.tile([P, 512], F32, tag="mm", name="mm", bufs=3)
                            for nb in range(NB):
                                nc.tensor.matmul(
                                    c_ps,
                                    expst[nb][:, msb * P:(msb + 1) * P],
                                    vts[nb][:, dc * 512:(dc + 1) * 512],
                                    start=(nb == 0),
                                    stop=(nb == NB - 1),
                                )
                            nc.vector.scalar_tensor_tensor(
                                out=ot[:, dc * 512:(dc + 1) * 512],
                                in0=c_ps,
                                scalar=recip_pp[:, msb:msb + 1],
                                in1=kn[:, dc * 512:(dc + 1) * 512],
                                op0=mybir.AluOpType.mult,
                                op1=mybir.AluOpType.add,
                            )
                        nc.scalar.dma_start(out=out[r0:r0 + P, :], in_=ot)

    nc.compile()
    return nc


def _get_nc():
    if "nc" not in _CACHE:
        _CACHE["nc"] = _build()
    return _CACHE["nc"]


def kernel(query_input, key_input, Wq, Wk, Wv):
    nc = _get_nc()
    bf = ml_dtypes.bfloat16
    query_input = np.asarray(query_input, dtype=np.float32)
    key_input = np.asarray(key_input, dtype=np.float32)
    Wq = np.asarray(Wq, dtype=np.float32)
    Wk = np.asarray(Wk, dtype=np.float32)
    Wv = np.asarray(Wv, dtype=np.float32)
    # weight pre-pack: gT = (Wq.T @ Wk).T = Wk.T @ Wq, so that
    # kgT = gT.T @ K.T on device with gT blocks as the stationary operand
    gT = np.ascontiguousarray(Wk.T @ Wq).astype(bf)
    wvT = np.ascontiguousarray(Wv.T).astype(bf)
    in_maps = []
    for b in range(B):
        in_maps.append({
            "xT": np.ascontiguousarray(query_input[b].T).astype(bf),
            "ktT": np.ascontiguousarray(key_input[b].T).astype(bf),
            "knat": np.ascontiguousarray(key_input[b]),
            "gT": gT,
            "wvT": wvT,
        })
    res = run_bass_kernel_spmd(nc, in_maps, list(range(B))).results
    return np.stack([res[b]["out"] for b in range(B)], axis=0)


# revision 3
# speedup vs baseline: 1.0827x; 1.0827x over previous
"""Trainium2 Bass kernel for nn_AttentionModule (dense transformer block).

Computation (per batch element b):
    q = X @ Wq.T ; k = K @ Wk.T ; v = X @ Wv.T        (X=query_input, K=key_input)
    a = softmax((k @ q.T) / sqrt(D), axis=-1)          -> (NK, NQ)
    out = a @ v + K                                    -> (NK, D)

Sharding: data-parallel over batch, one batch element per NeuronCore (B == 8).

q and k never appear individually -- only the Gram product k @ q.T does.
Folding the two projection weights into G = Wq.T @ Wk (host-side weight
pre-pack) turns the score matrix into S.T = X @ (G @ K.T), which removes the
entire q projection from the device: device work drops from 15.0 to 12.9
GMAC per core.

All matmuls run in fp8e4m3 with DoubleRow perf mode (2 contraction rows per
cell per cycle), accumulating in fp32 PSUM.  Operands live in 3D SBUF tiles
[128, n_sub, free] where dim 1 indexes contiguous 128-row contraction
blocks; each DoubleRow matmul consumes a [:, 2s:2s+2, :] slice (256-row
contraction step).  fp8 shrinks kg (= G @ K.T) and v to 2 MB each, so both
stay SBUF-resident between the projection and attention phases -- no DRAM
spill round-trips at all.  exp runs on the scalar engine writing fp8
directly; the softmax denominator is a ones-vector DoubleRow matmul over the
same fp8 exp tiles (numerator/denominator quantization errors partially
cancel), and the normalization is folded into the output pass as a fused
per-partition multiply-add on the vector engine in fp32.
"""

import numpy as np
import ml_dtypes

import concourse.tile as tile
from concourse import bacc, mybir
from concourse.bass_utils import run_bass_kernel_spmd
from concourse.masks import make_identity

B, NQ, NK, D = 8, 2048, 2048, 1024
P = 128
DBP = D // (2 * P)   # 4 contraction super-blocks (256 rows) over features
NBP = NQ // (2 * P)  # 8 contraction super-blocks over queries
NB = NQ // P         # 16 query-row blocks
MC = 512             # scores chunk width (n_k columns per chunk)
NMC = NK // MC       # 4 chunks
SCALE = 1.0 / float(np.sqrt(np.float32(D)))

F32 = mybir.dt.float32
F8 = mybir.dt.float8e4
DR = mybir.MatmulPerfMode.DoubleRow

_CACHE = {}


def _build():
    nc = bacc.Bacc("TRN2", target_bir_lowering=False, debug=False, num_devices=B)

    x8 = nc.dram_tensor("x8", [D, NQ], F8, kind="ExternalInput").ap()
    kt8 = nc.dram_tensor("kt8", [D, NK], F8, kind="ExternalInput").ap()
    knat = nc.dram_tensor("knat", [NK, D], F32, kind="ExternalInput").ap()
    g8 = nc.dram_tensor("g8", [D, D], F8, kind="ExternalInput").ap()
    wv8 = nc.dram_tensor("wv8", [D, D], F8, kind="ExternalInput").ap()
    out = nc.dram_tensor("out", [NK, D], F32, kind="ExternalOutput").ap()

    with tile.TileContext(nc) as tc:
        with (
            tc.tile_pool(name="const", bufs=1) as constp,
            tc.tile_pool(name="xin", bufs=DBP) as xinp,
            tc.tile_pool(name="kgp", bufs=DBP) as kgp,
            tc.tile_pool(name="vtp", bufs=NBP) as vtp,
            tc.tile_pool(name="psum", bufs=1, space="PSUM") as psp,
        ):
            ident = constp.tile([1, 1], F32, tag="ident", name="ident")
            make_identity(nc, ident)
            # DoubleRow stationary APs need dim-1 stride % 16 == 0, so the
            # ones vector is padded to 16 columns (output rows identical;
            # row 0 is consumed)
            ones = constp.tile([P, 2, 16], F8, tag="ones", name="ones")
            nc.vector.memset(ones, 1.0)

            # SBUF-resident across both phases:
            #   x_in[dbp] : X.T rows dbp*256..+255   (moving for v, stationary
            #               for scores)
            #   kgt[dbp]  : kg.T = (G @ K.T) rows dbp*256..+255  (moving for
            #               scores; filled by phase 1a)
            #   vts[nbp]  : v rows nbp*256..+255  (moving for context; filled
            #               by phase 1b straight from PSUM)
            x_in = [xinp.tile([P, 2, NQ], F8, tag="xin", name="xin")
                    for _ in range(DBP)]
            kgt = [kgp.tile([P, 2, NK], F8, tag="kg", name="kg")
                   for _ in range(DBP)]
            vts = [vtp.tile([P, 2, D], F8, tag="vt", name="vt")
                   for _ in range(NBP)]

            # ---------------- phase 1: projections ----------------
            with (
                tc.tile_pool(name="ktin", bufs=16) as ktp,
                tc.tile_pool(name="wpool", bufs=12) as wp,
            ):
                # loads in first-consumed order: g first half, ktT quarters,
                # g second half, then x / wv for the v projection.
                g_h = [[None] * 2 for _ in range(DBP)]
                kt_q = [[None] * 4 for _ in range(DBP)]
                for ebp in range(DBP):
                    t = wp.tile([P, 2, D // 2], F8, tag="gh", name="gh", bufs=8)
                    for s in range(2):
                        nc.sync.dma_start(
                            out=t[:, s, :],
                            in_=g8[ebp * 256 + s * P:ebp * 256 + (s + 1) * P, 0:512],
                        )
                    g_h[ebp][0] = t
                for q in range(4):
                    for ebp in range(DBP):
                        t = ktp.tile([P, 2, NK // 4], F8, tag="kth", name="kth",
                                     bufs=16)
                        for s in range(2):
                            nc.sync.dma_start(
                                out=t[:, s, :],
                                in_=kt8[ebp * 256 + s * P:ebp * 256 + (s + 1) * P,
                                        q * 512:(q + 1) * 512],
                            )
                        kt_q[ebp][q] = t
                for ebp in range(DBP):
                    t = wp.tile([P, 2, D // 2], F8, tag="gh", name="gh", bufs=8)
                    for s in range(2):
                        nc.sync.dma_start(
                            out=t[:, s, :],
                            in_=g8[ebp * 256 + s * P:ebp * 256 + (s + 1) * P,
                                   512:1024],
                        )
                    g_h[ebp][1] = t

                # -- kg.T[d, m] = sum_e gT[e, d] * K.T[e, m]
                # (gT = G.T = Wk.T @ Wq supplied by host; output row-block db
                #  lands in kgt[db//2][:, db%2, :])
                gi = 0
                for h2 in range(2):
                    for mc4 in range(NK // 512):
                        for db in range(h2 * 4, h2 * 4 + 4):
                            tg = "mm" if gi % 2 == 0 else "st"
                            gi += 1
                            ps = psp.tile([P, 512], F32, tag=tg, name="mm",
                                          bufs=3 if tg == "mm" else 4)
                            for ebp in range(DBP):
                                nc.tensor.matmul(
                                    ps,
                                    g_h[ebp][h2][:, :, (db % 4) * P:(db % 4 + 1) * P],
                                    kt_q[ebp][mc4],
                                    start=(ebp == 0),
                                    stop=(ebp == DBP - 1),
                                    perf_mode=DR,
                                )
                            nc.vector.tensor_copy(
                                kgt[db // 2][:, db % 2, mc4 * 512:(mc4 + 1) * 512],
                                ps,
                            )

                # -- v[n, dv] = sum_d X.T[d, n] * Wv.T[d, dv]
                # (output row-block nb lands in vts[nb//2][:, nb%2, :])
                for dbp in range(DBP):
                    for s in range(2):
                        nc.sync.dma_start(
                            out=x_in[dbp][:, s, :],
                            in_=x8[dbp * 256 + s * P:dbp * 256 + (s + 1) * P, :],
                        )
                wv = []
                for dbp in range(DBP):
                    t = wp.tile([P, 2, D], F8, tag="w", name="w", bufs=4)
                    for s in range(2):
                        nc.sync.dma_start(
                            out=t[:, s, :],
                            in_=wv8[dbp * 256 + s * P:dbp * 256 + (s + 1) * P, :],
                        )
                    wv.append(t)
                for nb in range(NB):
                    for dc in range(D // 512):
                        tg = "mm" if (nb * 2 + dc) % 2 == 0 else "st"
                        ps = psp.tile([P, 512], F32, tag=tg, name="mm",
                                      bufs=3 if tg == "mm" else 4)
                        for dbp in range(DBP):
                            nc.tensor.matmul(
                                ps,
                                x_in[dbp][:, :, nb * P:(nb + 1) * P],
                                wv[dbp][:, :, dc * 512:(dc + 1) * 512],
                                start=(dbp == 0),
                                stop=(dbp == DBP - 1),
                                perf_mode=DR,
                            )
                        nc.vector.tensor_copy(
                            vts[nb // 2][:, nb % 2, dc * 512:(dc + 1) * 512],
                            ps,
                        )

            # ---------------- phase 2: attention ----------------
            with (
                tc.tile_pool(name="expst", bufs=10) as expp,
                tc.tile_pool(name="knp", bufs=6) as knp,
                tc.tile_pool(name="outp", bufs=6) as outp,
                tc.tile_pool(name="small", bufs=4) as smallp,
            ):
                for mc in range(NMC):
                    m0 = mc * MC

                    # scores + exp + column-sum accumulation
                    # exp of row-block nb lands in expst[nb//2][:, nb%2, :]
                    expst = [expp.tile([P, 2, MC], F8, tag="expst", name="expst")
                             for _ in range(NBP)]
                    cs_ps = psp.tile([16, MC], F32, tag="csrp", name="cs", bufs=1)
                    for nb in range(NB):
                        st_ps = psp.tile([P, MC], F32, tag="st", name="st", bufs=4)
                        for dbp in range(DBP):
                            nc.tensor.matmul(
                                st_ps,
                                x_in[dbp][:, :, nb * P:(nb + 1) * P],
                                kgt[dbp][:, :, m0:m0 + MC],
                                start=(dbp == 0),
                                stop=(dbp == DBP - 1),
                                perf_mode=DR,
                            )
                        nc.scalar.activation(
                            out=expst[nb // 2][:, nb % 2, :], in_=st_ps,
                            func=mybir.ActivationFunctionType.Exp, scale=SCALE,
                        )
                        # the column-sum matmul for pair j is emitted two
                        # score-groups late so the exp -> cs semaphore never
                        # gates PE
                        if nb >= 3 and nb % 2 == 1:
                            j = (nb - 3) // 2
                            nc.tensor.matmul(
                                cs_ps, ones, expst[j],
                                start=(j == 0), stop=False, perf_mode=DR,
                            )
                    nc.tensor.matmul(
                        cs_ps, ones, expst[NBP - 1],
                        start=False, stop=True, perf_mode=DR,
                    )
                    recip_row = smallp.tile([1, MC], F32, tag="rrow", name="rrow")
                    nc.vector.reciprocal(recip_row, cs_ps[0:1, :])
                    rp_ps = psp.tile([P, MC // P], F32, tag="csrp", name="rp", bufs=1)
                    for j in range(MC // P):
                        nc.tensor.transpose(
                            rp_ps[:, j:j + 1],
                            recip_row[:, j * P:(j + 1) * P],
                            ident,
                        )
                    recip_pp = smallp.tile([P, MC // P], F32, tag="rpp", name="rpp")
                    nc.vector.tensor_copy(recip_pp, rp_ps)

                    # context: C[m, dv] = sum_n expst[n, m] * v[n, dv]
                    for msb in range(MC // P):
                        r0 = m0 + msb * P
                        kn = knp.tile([P, D], F32, tag="knat", name="knat")
                        nc.sync.dma_start(out=kn, in_=knat[r0:r0 + P, :])
                        ot = outp.tile([P, D], F32, tag="ostage", name="ostage")
                        for dc in range(D // 512):
                            c_ps = psp.tile([P, 512], F32, tag="mm", name="mm", bufs=3)
                            for nbp in range(NBP):
                                nc.tensor.matmul(
                                    c_ps,
                                    expst[nbp][:, :, msb * P:(msb + 1) * P],
                                    vts[nbp][:, :, dc * 512:(dc + 1) * 512],
                                    start=(nbp == 0),
                                    stop=(nbp == NBP - 1),
                                    perf_mode=DR,
                                )
                            nc.vector.scalar_tensor_tensor(
                                out=ot[:, dc * 512:(dc + 1) * 512],
                                in0=c_ps,
                                scalar=recip_pp[:, msb:msb + 1],
                                in1=kn[:, dc * 512:(dc + 1) * 512],
                                op0=mybir.AluOpType.mult,
                                op1=mybir.AluOpType.add,
                            )
                        nc.scalar.dma_start(out=out[r0:r0 + P, :], in_=ot)

    nc.compile()
    return nc


def _get_nc():
    if "nc" not in _CACHE:
        _CACHE["nc"] = _build()
    return _CACHE["nc"]


def _prep_in_maps(query_input, key_input, Wq, Wk, Wv):
    f8 = ml_dtypes.float8_e4m3
    query_input = np.asarray(query_input, dtype=np.float32)
    key_input = np.asarray(key_input, dtype=np.float32)
    Wq = np.asarray(Wq, dtype=np.float32)
    Wk = np.asarray(Wk, dtype=np.float32)
    Wv = np.asarray(Wv, dtype=np.float32)
    # weight pre-pack: g8 = G.T = (Wq.T @ Wk).T = Wk.T @ Wq, so that
    # kg.T = g8.T @ K.T on device with g8 blocks as the stationary operand
    g8 = np.ascontiguousarray(Wk.T @ Wq).astype(f8)
    wv8 = np.ascontiguousarray(Wv.T).astype(f8)
    in_maps = []
    for b in range(B):
        in_maps.append({
            "x8": np.ascontiguousarray(query_input[b].T).astype(f8),
            "kt8": np.ascontiguousarray(key_input[b].T).astype(f8),
            "knat": np.ascontiguousarray(key_input[b]),
            "g8": g8,
            "wv8": wv8,
        })
    return in_maps


def kernel(query_input, key_input, Wq, Wk, Wv):
    nc = _get_nc()
    in_maps = _prep_in_maps(query_input, key_input, Wq, Wk, Wv)
    res = run_bass_kernel_spmd(nc, in_maps, list(range(B))).results
    return np.stack([res[b]["out"] for b in range(B)], axis=0)


# revision 4
# speedup vs baseline: 1.1216x; 1.0359x over previous
"""Trainium2 Bass kernel for nn_AttentionModule (dense transformer block).

Computation (per batch element b):
    q = X @ Wq.T ; k = K @ Wk.T ; v = X @ Wv.T        (X=query_input, K=key_input)
    a = softmax((k @ q.T) / sqrt(D), axis=-1)          -> (NK, NQ)
    out = a @ v + K                                    -> (NK, D)

Sharding: data-parallel over batch, one batch element per NeuronCore (B == 8).

q and k never appear individually -- only the Gram product k @ q.T does.
Folding the two projection weights into G = Wq.T @ Wk (host-side weight
pre-pack) turns the score matrix into S.T = X @ (G @ K.T), which removes the
entire q projection from the device: device work drops from 15.0 to 12.9
GMAC per core.

All matmuls run in fp8e4m3 with DoubleRow perf mode (2 contraction rows per
cell per cycle), accumulating in fp32 PSUM.  Every operand lives in a single
3D SBUF tile [128, n_sub, cols] where dim 1 indexes contiguous 128-row
blocks of the contraction axis; a DoubleRow matmul consumes a
[:, 2k:2k+2, :] slice (256-row contraction step).  This layout lets each
DRAM tensor load with one or two large rearranged DMA descriptors (the
~1 us per-descriptor DGE overhead otherwise dominates the fill), and fp8
shrinks kg (= G @ K.T) and v to 2 MB each so both stay SBUF-resident
between phases -- no DRAM spill round-trips.  PSUM evacuation copies
alternate between the vector and scalar engines so neither gates the PE.
exp runs on the scalar engine writing fp8 directly; the softmax denominator
is a ones-vector DoubleRow matmul over the same fp8 exp tiles (numerator /
denominator quantization errors partially cancel), and the normalization is
folded into the output pass as a fused per-partition multiply-add on the
vector engine in fp32.
"""

import numpy as np
import ml_dtypes

import concourse.tile as tile
from concourse import bacc, mybir
from concourse.bass_utils import run_bass_kernel_spmd
from concourse.masks import make_identity

B, NQ, NK, D = 8, 2048, 2048, 1024
P = 128
DSB = D // P         # 8 feature sub-blocks
NSB = NQ // P        # 16 query sub-blocks
DBP = DSB // 2       # 4 DoubleRow steps over features
NBP = NSB // 2       # 8 DoubleRow steps over queries
NB = NQ // P         # 16 query-row blocks
MC = 512             # scores chunk width (n_k columns per chunk)
NMC = NK // MC       # 4 chunks
SCALE = 1.0 / float(np.sqrt(np.float32(D)))

F32 = mybir.dt.float32
F8 = mybir.dt.float8e4
DR = mybir.MatmulPerfMode.DoubleRow

_CACHE = {}


def _sub(ap):
    """[R, C] dram AP -> [128, R//128, C] with dim 1 = contiguous row blocks."""
    return ap.rearrange("(s p) n -> p s n", p=P)


def _build():
    nc = bacc.Bacc("TRN2", target_bir_lowering=False, debug=False, num_devices=B)

    x8 = nc.dram_tensor("x8", [D, NQ], F8, kind="ExternalInput").ap()
    kt8 = nc.dram_tensor("kt8", [D, NK], F8, kind="ExternalInput").ap()
    knat = nc.dram_tensor("knat", [NK, D], F32, kind="ExternalInput").ap()
    g8 = nc.dram_tensor("g8", [D, D], F8, kind="ExternalInput").ap()
    wv8 = nc.dram_tensor("wv8", [D, D], F8, kind="ExternalInput").ap()
    out = nc.dram_tensor("out", [NK, D], F32, kind="ExternalOutput").ap()

    with tile.TileContext(nc) as tc:
        with (
            tc.tile_pool(name="const", bufs=1) as constp,
            tc.tile_pool(name="big", bufs=1) as bigp,
            tc.tile_pool(name="psum", bufs=1, space="PSUM") as psp,
        ):
            ident = constp.tile([1, 1], F32, tag="ident", name="ident")
            make_identity(nc, ident)
            # DoubleRow stationary APs need dim-1 stride % 16 == 0, so the
            # ones vector is padded to 16 columns (output rows identical;
            # row 0 is consumed)
            ones = constp.tile([P, 2, 16], F8, tag="ones", name="ones")
            nc.vector.memset(ones, 1.0)

            # input operands as per-column-block 3D tiles, one large
            # rearranged DMA each (Tile's dependency tracking is per-tile
            # coarse-interval, so a consumer of one tile never waits on
            # another block's load); kgall/vall are filled from PSUM by
            # phase 1 and consumed by phase 2 (SBUF-resident, no DRAM spill)
            g_h = [bigp.tile([P, DSB, D // 2], F8, tag=f"g{h}", name="gh")
                   for h in range(2)]
            kt_q = [bigp.tile([P, DSB, NK // 4], F8, tag=f"kt{q}", name="ktq")
                    for q in range(4)]
            x_q = [bigp.tile([P, DSB, NQ // 4], F8, tag=f"x{q}", name="xq")
                   for q in range(4)]
            wvall = bigp.tile([P, DSB, D], F8, tag="wvall", name="wvall")
            kgall = bigp.tile([P, DSB, NK], F8, tag="kgall", name="kgall")
            vall = bigp.tile([P, NSB, D], F8, tag="vall", name="vall")

            # loads in first-consumed order; the kt quarters alternate
            # between the gpsimd and sync DGE rings so the stream keeps up
            # with phase-1a consumption, g/wv lead on sync, x follows on
            # both rings for phase 1b
            nc.sync.dma_start(out=g_h[0], in_=_sub(g8[:, 0:D // 2]))
            nc.gpsimd.dma_start(out=kt_q[0], in_=_sub(kt8[:, 0:512]))
            nc.sync.dma_start(out=g_h[1], in_=_sub(g8[:, D // 2:D]))
            nc.sync.dma_start(out=kt_q[1], in_=_sub(kt8[:, 512:1024]))
            nc.gpsimd.dma_start(out=kt_q[2], in_=_sub(kt8[:, 1024:1536]))
            nc.sync.dma_start(out=kt_q[3], in_=_sub(kt8[:, 1536:2048]))
            nc.gpsimd.dma_start(out=x_q[0], in_=_sub(x8[:, 0:512]))
            nc.sync.dma_start(out=wvall, in_=_sub(wv8))
            nc.gpsimd.dma_start(out=x_q[1], in_=_sub(x8[:, 512:1024]))
            nc.gpsimd.dma_start(out=x_q[2], in_=_sub(x8[:, 1024:1536]))
            nc.gpsimd.dma_start(out=x_q[3], in_=_sub(x8[:, 1536:2048]))

            # ---------------- phase 1: projections ----------------
            # -- kg.T[d, m] = sum_e gT[e, d] * K.T[e, m]
            # (gT = G.T = Wk.T @ Wq supplied by host; output row-block db
            #  lands in kgall[:, db, :])
            gi = 0
            for mc4 in range(NK // 512):
                for db in range(DSB):
                    tg = "mm" if gi % 2 == 0 else "st"
                    ps = psp.tile([P, 512], F32, tag=tg, name="mm", bufs=2 if tg == "mm" else 4)
                    for ebp in range(DBP):
                        nc.tensor.matmul(
                            ps,
                            g_h[db // 4][:, 2 * ebp:2 * ebp + 2,
                                         (db % 4) * P:(db % 4 + 1) * P],
                            kt_q[mc4][:, 2 * ebp:2 * ebp + 2, :],
                            start=(ebp == 0),
                            stop=(ebp == DBP - 1),
                            perf_mode=DR,
                        )
                    dst = kgall[:, db, mc4 * 512:(mc4 + 1) * 512]
                    if gi % 2 == 0:
                        nc.vector.tensor_copy(dst, ps)
                    else:
                        nc.scalar.copy(dst, ps)
                    gi += 1

            # -- v[n, dv] = sum_d X.T[d, n] * Wv.T[d, dv]
            # (output row-block nb lands in vall[:, nb, :])
            for nb in range(NB):
                for dc in range(D // 512):
                    gi += 1
                    tg = "mm" if gi % 2 == 0 else "st"
                    ps = psp.tile([P, 512], F32, tag=tg, name="mm", bufs=2 if tg == "mm" else 4)
                    for dbp in range(DBP):
                        nc.tensor.matmul(
                            ps,
                            x_q[nb // 4][:, 2 * dbp:2 * dbp + 2,
                                         (nb % 4) * P:(nb % 4 + 1) * P],
                            wvall[:, 2 * dbp:2 * dbp + 2, dc * 512:(dc + 1) * 512],
                            start=(dbp == 0),
                            stop=(dbp == DBP - 1),
                            perf_mode=DR,
                        )
                    dst = vall[:, nb, dc * 512:(dc + 1) * 512]
                    if gi % 2 == 0:
                        nc.vector.tensor_copy(dst, ps)
                    else:
                        nc.scalar.copy(dst, ps)

            # ---------------- phase 2: attention ----------------
            with (
                tc.tile_pool(name="expst", bufs=2) as expp,
                tc.tile_pool(name="knp", bufs=2) as knp,
                tc.tile_pool(name="outp", bufs=6) as outp,
                tc.tile_pool(name="small", bufs=4) as smallp,
            ):
                for mc in range(NMC):
                    m0 = mc * MC

                    # residual rows for this chunk: one big rearranged load
                    knt = knp.tile([P, 4, D], F32, tag="knat", name="knat")
                    nc.sync.dma_start(out=knt, in_=_sub(knat[m0:m0 + MC, :]))

                    # scores + exp + column-sum accumulation
                    # exp of row-block nb lands in expst[:, nb, :]
                    expst = expp.tile([P, NSB, MC], F8, tag="expst", name="expst")
                    cs_ps = psp.tile([16, MC], F32, tag="csrp", name="cs", bufs=2)
                    for nb in range(NB):
                        st_ps = psp.tile([P, MC], F32, tag="st", name="st", bufs=4)
                        for dbp in range(DBP):
                            nc.tensor.matmul(
                                st_ps,
                                x_q[nb // 4][:, 2 * dbp:2 * dbp + 2,
                                             (nb % 4) * P:(nb % 4 + 1) * P],
                                kgall[:, 2 * dbp:2 * dbp + 2, m0:m0 + MC],
                                start=(dbp == 0),
                                stop=(dbp == DBP - 1),
                                perf_mode=DR,
                            )
                        nc.scalar.activation(
                            out=expst[:, nb, :], in_=st_ps,
                            func=mybir.ActivationFunctionType.Exp, scale=SCALE,
                        )
                        # the column-sum matmul for pair j is emitted two
                        # score-groups late so the exp -> cs semaphore never
                        # gates PE
                        if nb >= 3 and nb % 2 == 1:
                            j = (nb - 3) // 2
                            nc.tensor.matmul(
                                cs_ps, ones, expst[:, 2 * j:2 * j + 2, :],
                                start=(j == 0), stop=False, perf_mode=DR,
                            )
                    nc.tensor.matmul(
                        cs_ps, ones, expst[:, NSB - 2:NSB, :],
                        start=False, stop=True, perf_mode=DR,
                    )
                    recip_row = smallp.tile([1, MC], F32, tag="rrow", name="rrow")
                    nc.vector.reciprocal(recip_row, cs_ps[0:1, :])
                    rp_ps = psp.tile([P, MC // P], F32, tag="csrp", name="rp", bufs=2)
                    for j in range(MC // P):
                        nc.tensor.transpose(
                            rp_ps[:, j:j + 1],
                            recip_row[:, j * P:(j + 1) * P],
                            ident,
                        )
                    recip_pp = smallp.tile([P, MC // P], F32, tag="rpp", name="rpp")
                    nc.vector.tensor_copy(recip_pp, rp_ps)

                    # context: C[m, dv] = sum_n expst[n, m] * v[n, dv]
                    for msb in range(MC // P):
                        r0 = m0 + msb * P
                        ot = outp.tile([P, D], F32, tag="ostage", name="ostage")
                        for dc in range(D // 512):
                            c_ps = psp.tile([P, 512], F32, tag="mm", name="mm", bufs=2)
                            for nbp in range(NBP):
                                nc.tensor.matmul(
                                    c_ps,
                                    expst[:, 2 * nbp:2 * nbp + 2,
                                          msb * P:(msb + 1) * P],
                                    vall[:, 2 * nbp:2 * nbp + 2,
                                         dc * 512:(dc + 1) * 512],
                                    start=(nbp == 0),
                                    stop=(nbp == NBP - 1),
                                    perf_mode=DR,
                                )
                            nc.vector.scalar_tensor_tensor(
                                out=ot[:, dc * 512:(dc + 1) * 512],
                                in0=c_ps,
                                scalar=recip_pp[:, msb:msb + 1],
                                in1=knt[:, msb, dc * 512:(dc + 1) * 512],
                                op0=mybir.AluOpType.mult,
                                op1=mybir.AluOpType.add,
                            )
                        nc.scalar.dma_start(out=out[r0:r0 + P, :], in_=ot)

    nc.compile()
    return nc


def _get_nc():
    if "nc" not in _CACHE:
        _CACHE["nc"] = _build()
    return _CACHE["nc"]


def _prep_in_maps(query_input, key_input, Wq, Wk, Wv):
    f8 = ml_dtypes.float8_e4m3
    query_input = np.asarray(query_input, dtype=np.float32)
    key_input = np.asarray(key_input, dtype=np.float32)
    Wq = np.asarray(Wq, dtype=np.float32)
    Wk = np.asarray(Wk, dtype=np.float32)
    Wv = np.asarray(Wv, dtype=np.float32)
    # weight pre-pack: g8 = G.T = (Wq.T @ Wk).T = Wk.T @ Wq, so that
    # kg.T = g8.T @ K.T on device with g8 blocks as the stationary operand
    g8 = np.ascontiguousarray(Wk.T @ Wq).astype(f8)
    wv8 = np.ascontiguousarray(Wv.T).astype(f8)
    in_maps = []
    for b in range(B):
        in_maps.append({
            "x8": np.ascontiguousarray(query_input[b].T).astype(f8),
            "kt8": np.ascontiguousarray(key_input[b].T).astype(f8),
            "knat": np.ascontiguousarray(key_input[b]),
            "g8": g8,
            "wv8": wv8,
        })
    return in_maps


def kernel(query_input, key_input, Wq, Wk, Wv):
    nc = _get_nc()
    in_maps = _prep_in_maps(query_input, key_input, Wq, Wk, Wv)
    res = run_bass_kernel_spmd(nc, in_maps, list(range(B))).results
    return np.stack([res[b]["out"] for b in range(B)], axis=0)
